# revision 11
# baseline (speedup 1.0000x reference)
"""Trainium2 Bass kernel for ComplexMoE (E=4 experts, top-2 routing).

Strategy: EXPERT-PARALLEL with host-side token dispatch (per the
sharding hint: "all-to-all dispatch tokens by top-k expert id").
The tiny router (8192x1024x4 matmul + top-2 + softmax) runs on the host
as part of sharding; each of the 8 cores owns ONE expert's weights
(2 cores per expert, each taking up to 2048 of that expert's routed
tokens). A core computes only its ~2048 dispatched token-slots instead
of all 4 experts x 1024 tokens densely: 2x less matmul work, and each
core streams only its own expert's weights. The handful of tokens that
overflow the 2x2048 per-expert device capacity (expected count ~40 of
16384 slots for balanced routing) are computed exactly on the host in
fp32 and added during the combine - capacity is a perf knob, not a
correctness cliff.

Complex matmuls use the 3-multiplication Karatsuba/Gauss form (25%
fewer matmuls); its operand sums are free on the host:
  k1 = W1 @ (ar+ai), k2 = W2 @ ar, k3 = W3 @ ai
  with W1 = wr, W2 = wi-wr, W3 = wr+wi   (host-precomputed)
  yr = k1 - k3 ; yi = k1 + k2
Matmuls run in bf16 (same 1 cycle/row PE rate as f32r, with FWL hiding
the weight loads and half the DMA bytes); accumulation stays fp32 in
PSUM. The top-2 softmax combine weight w folds into the value-path
activations on the host (xv = w*x), so h = silu(|g|) * v_scaled needs
no on-device broadcast or extra multiply:
  y = down(silu(|g(x)|) * v(w*x)) = w * y_expert   (v, down linear).

Per-core device program (SPMD, no collectives, 4 chunks of 512 slots):
  for ch in chunks:
    for j in 8 dh-tiles:   # up: ug and uv complex matmuls
      6 PSUM banks (G1..3,V1..3) <- 24 bf16 matmuls (bank-grouped
      emission so PSUM evacuation never stalls the PE)
      gate: DVE subs/adds from PSUM + ACT square/sqrt/silu ordered so
      neither engine head-of-line blocks; hr/hi/hs written bf16
    for d in 4 d-tiles:    # down complex matmul
      3 PSUM banks <- 24 bf16 matmuls; DVE recombine -> DMA out
"""

import numpy as np
import ml_dtypes

import concourse.bacc as bacc
import concourse.bass as bass
import concourse.mybir as mybir
import concourse.tile as tile
from concourse.bass_utils import run_bass_kernel_spmd

B, H, T, D = 2, 8, 512, 512
DH = 1024
E = 4
K = 2
NCORES = 8
NTOK = B * H * T            # 8192
KD = D // 128               # 4 k-tiles over D
KH = DH // 128              # 8 k-tiles over DH
CHW = 512                   # chunk width (one fp32 psum bank)
NCH = 4
CAP = NCH * CHW             # 2048 device slots per core

f32 = mybir.dt.float32
bf16 = mybir.dt.bfloat16
ACT = mybir.ActivationFunctionType
ALU = mybir.AluOpType
np_bf16 = ml_dtypes.bfloat16


def _build_bass():
    nc = bacc.Bacc(None)

    # gate-path x (raw) and value-path x (combine-weight-scaled),
    # chunk-major feature-major: [NCH, 128, KD, CHW]
    xgr = nc.declare_dram_parameter("xgr", [NCH, 128, KD, CHW], bf16,
                                    isOutput=False)
    xgi = nc.declare_dram_parameter("xgi", [NCH, 128, KD, CHW], bf16,
                                    isOutput=False)
    xgs = nc.declare_dram_parameter("xgs", [NCH, 128, KD, CHW], bf16,
                                    isOutput=False)
    xvr = nc.declare_dram_parameter("xvr", [NCH, 128, KD, CHW], bf16,
                                    isOutput=False)
    xvi = nc.declare_dram_parameter("xvi", [NCH, 128, KD, CHW], bf16,
                                    isOutput=False)
    xvs = nc.declare_dram_parameter("xvs", [NCH, 128, KD, CHW], bf16,
                                    isOutput=False)
    # this core's expert weights, Karatsuba triples stacked innermost:
    # up: (ug_W1, ug_W2, ug_W3, uv_W1, uv_W2, uv_W3); dn: (W1, W2, W3)
    upw = nc.declare_dram_parameter("upw", [KH, 128, KD, 6, 128], bf16,
                                    isOutput=False)
    dnw = nc.declare_dram_parameter("dnw", [KD, 128, KH, 3, 128], bf16,
                                    isOutput=False)
    oyr = nc.declare_dram_parameter("oyr", [NCH, 128, KD, CHW], f32,
                                    isOutput=True)
    oyi = nc.declare_dram_parameter("oyi", [NCH, 128, KD, CHW], f32,
                                    isOutput=True)

    with tile.TileContext(nc) as tc:
        with (
            tc.tile_pool(name="xp", bufs=2) as xp,
            tc.tile_pool(name="hp", bufs=1) as hp,
            tc.tile_pool(name="gt", bufs=2) as gt,
            tc.tile_pool(name="vp", bufs=2) as vp,
            tc.tile_pool(name="wup", bufs=3) as wup,
            tc.tile_pool(name="wdn", bufs=3) as wdn,
            tc.tile_pool(name="outp", bufs=2) as outp,
            tc.tile_pool(name="smalls", bufs=1) as smalls,
            tc.tile_pool(name="psA", bufs=1, space="PSUM") as psA,
            tc.tile_pool(name="psB", bufs=2, space="PSUM") as psB,
        ):
            epsb = smalls.tile([128, 1], f32, tag="epsb")
            nc.vector.memset(epsb, 1e-8)

            def load_x(ch, eng):
                xtr = xp.tile([128, KD, CHW], bf16, tag="xtr")
                xti = xp.tile([128, KD, CHW], bf16, tag="xti")
                xts = xp.tile([128, KD, CHW], bf16, tag="xts")
                xwr = xp.tile([128, KD, CHW], bf16, tag="xwr")
                xwi = xp.tile([128, KD, CHW], bf16, tag="xwi")
                xws = xp.tile([128, KD, CHW], bf16, tag="xws")
                eng.dma_start(out=xts, in_=xgs[ch])
                eng.dma_start(out=xtr, in_=xgr[ch])
                eng.dma_start(out=xti, in_=xgi[ch])
                eng.dma_start(out=xws, in_=xvs[ch])
                eng.dma_start(out=xwr, in_=xvr[ch])
                eng.dma_start(out=xwi, in_=xvi[ch])
                return xtr, xti, xts, xwr, xwi, xws

            cur_x = load_x(0, nc.sync)
            for ch in range(NCH):
                xtr, xti, xts, xwr, xwi, xws = cur_x

                hr = hp.tile([128, KH, CHW], bf16, tag="hr")
                hi = hp.tile([128, KH, CHW], bf16, tag="hi")
                hs = hp.tile([128, KH, CHW], bf16, tag="hs")

                # -------- up projections + gate, per dh-tile --------
                for j in range(KH):
                    uw = wup.tile([128, KD, 6, 128], bf16, tag="uw")
                    nc.sync.dma_start(out=uw, in_=upw[j])
                    G1 = psB.tile([128, CHW], f32, tag="pg1")
                    G2 = psA.tile([128, CHW], f32, tag="pg2")
                    G3 = psA.tile([128, CHW], f32, tag="pg3")
                    V1 = psB.tile([128, CHW], f32, tag="pv1")
                    V2 = psA.tile([128, CHW], f32, tag="pv2")
                    V3 = psA.tile([128, CHW], f32, tag="pv3")
                    # bank-grouped emission: all G matmuls, then all V,
                    # so PSUM evacuation of G overlaps the V matmuls.
                    for k in range(KD):
                        st, sp = (k == 0), (k == KD - 1)
                        nc.tensor.matmul(G1, uw[:, k, 0, :], xts[:, k, :],
                                         start=st, stop=sp)
                        nc.tensor.matmul(G2, uw[:, k, 1, :], xtr[:, k, :],
                                         start=st, stop=sp)
                        nc.tensor.matmul(G3, uw[:, k, 2, :], xti[:, k, :],
                                         start=st, stop=sp)
                    for k in range(KD):
                        st, sp = (k == 0), (k == KD - 1)
                        nc.tensor.matmul(V1, uw[:, k, 3, :], xws[:, k, :],
                                         start=st, stop=sp)
                        nc.tensor.matmul(V2, uw[:, k, 4, :], xwr[:, k, :],
                                         start=st, stop=sp)
                        nc.tensor.matmul(V3, uw[:, k, 5, :], xwi[:, k, :],
                                         start=st, stop=sp)
                    # gate = silu(sqrt(gr^2+gi^2+eps)); h = gate * v
                    g1c = gt.tile([128, CHW], f32, tag="g1c")
                    v1c = gt.tile([128, CHW], f32, tag="v1c")
                    gr = gt.tile([128, CHW], f32, tag="gr")
                    gi = gt.tile([128, CHW], f32, tag="gi")
                    t1 = gt.tile([128, CHW], f32, tag="t1")
                    t2 = gt.tile([128, CHW], f32, tag="t2")
                    t3 = gt.tile([128, CHW], f32, tag="t3")
                    m = gt.tile([128, CHW], f32, tag="m")
                    sm = gt.tile([128, CHW], f32, tag="sm")
                    vr = vp.tile([128, CHW], f32, tag="vr")
                    vi = vp.tile([128, CHW], f32, tag="vi")
                    # DVE reads at most one PSUM operand: stage G1/V1 to
                    # SBUF on the scalar engine. Emission order keeps the
                    # critical chain gr->t1->t2->t3->m->sm->hr/hi/hs
                    # flowing without head-of-line blocks on either queue.
                    nc.scalar.copy(out=g1c, in_=G1)
                    nc.vector.tensor_tensor(gr, g1c, G3, op=ALU.subtract)
                    nc.vector.tensor_tensor(gi, g1c, G2, op=ALU.add)
                    nc.scalar.activation(out=t1, in_=gr, func=ACT.Square)
                    nc.scalar.activation(out=t2, in_=gi, func=ACT.Square)
                    nc.vector.tensor_tensor(t3, t1, t2, op=ALU.add)
                    nc.scalar.copy(out=v1c, in_=V1)
                    nc.scalar.activation(out=m, in_=t3, func=ACT.Sqrt,
                                         bias=epsb)
                    nc.scalar.activation(out=sm, in_=m, func=ACT.Silu)
                    nc.vector.tensor_tensor(vr, v1c, V3, op=ALU.subtract)
                    nc.vector.tensor_tensor(vi, v1c, V2, op=ALU.add)
                    nc.vector.tensor_tensor(hr[:, j, :], sm, vr, op=ALU.mult)
                    nc.vector.tensor_tensor(hi[:, j, :], sm, vi, op=ALU.mult)
                    nc.vector.tensor_tensor(hs[:, j, :], hr[:, j, :],
                                            hi[:, j, :], op=ALU.add)

                # prefetch next chunk's tokens during the down phase
                # (scalar-engine HWDGE ring, so it doesn't queue behind
                # the weight streams on the sync ring)
                if ch + 1 < NCH:
                    cur_x = load_x(ch + 1, nc.scalar)

                # -------- down projection, per d-tile --------
                for d in range(KD):
                    dw = wdn.tile([128, KH, 3, 128], bf16, tag="dw")
                    nc.sync.dma_start(out=dw, in_=dnw[d])
                    D1 = psB.tile([128, CHW], f32, tag="pg1")
                    D2 = psA.tile([128, CHW], f32, tag="pg2")
                    D3 = psA.tile([128, CHW], f32, tag="pg3")
                    for kh in range(KH):
                        nc.tensor.matmul(D1, dw[:, kh, 0, :], hs[:, kh, :],
                                         start=(kh == 0), stop=(kh == KH - 1))
                    for kh in range(KH):
                        nc.tensor.matmul(D2, dw[:, kh, 1, :], hr[:, kh, :],
                                         start=(kh == 0), stop=(kh == KH - 1))
                    for kh in range(KH):
                        nc.tensor.matmul(D3, dw[:, kh, 2, :], hi[:, kh, :],
                                         start=(kh == 0), stop=(kh == KH - 1))
                    d1c = outp.tile([128, CHW], f32, tag="d1c")
                    yr = outp.tile([128, CHW], f32, tag="yr")
                    yi = outp.tile([128, CHW], f32, tag="yi")
                    nc.scalar.copy(out=d1c, in_=D1)
                    nc.vector.tensor_tensor(yr, d1c, D3, op=ALU.subtract)
                    nc.vector.tensor_tensor(yi, d1c, D2, op=ALU.add)
                    nc.sync.dma_start(out=oyr[ch, :, d, :], in_=yr)
                    nc.scalar.dma_start(out=oyi[ch, :, d, :], in_=yi)
    nc.finalize()
    return nc


_cached_nc = None


def _get_nc():
    global _cached_nc
    if _cached_nc is None:
        _cached_nc = _build_bass()
    return _cached_nc


def _route(xr2, xi2, router_w, router_b):
    """Host router: fp32 logits, stable top-2, softmax weights."""
    logits = (xr2 @ router_w[:, :D].T.astype(np.float32)
              + xi2 @ router_w[:, D:].T.astype(np.float32)
              + router_b[None, :].astype(np.float32))
    order = np.argsort(-logits, axis=1, kind="stable")
    top2 = order[:, :K]                                   # [N, 2]
    l12 = np.take_along_axis(logits, top2, axis=1)
    l12 = l12 - l12[:, :1]                                # max first (sorted)
    ew = np.exp(l12)
    wts = ew / ew.sum(axis=1, keepdims=True)              # [N, 2]
    return top2, wts.astype(np.float32)


def _dispatch(top2, wts):
    """Assign each (token, expert) pair to a device slot or the host.

    Expert e lives on cores 2e and 2e+1 (up to CAP slots each); pairs
    beyond 2*CAP go to the host overflow list (computed exactly in
    numpy). Returns per-core index/weight arrays, the [N, 2] global
    slot map (-1 = host), and the overflow list [(e, tokens, weights)].
    """
    N = top2.shape[0]
    core_idx = [None] * NCORES
    core_wts = [None] * NCORES
    slotmap = np.empty((N, K), np.int64)
    overflow = []
    for e in range(E):
        tok_e, which = np.nonzero(top2 == e)
        c_e = len(tok_e)
        n1 = min((c_e + 1) // 2, CAP)
        n2 = min(c_e - n1, CAP)
        nd = n1 + n2
        pos = np.arange(c_e)
        in2 = pos >= n1
        cores = 2 * e + in2.astype(np.int64)
        pic = np.where(in2, pos - n1, pos)
        gslot = cores * CAP + pic
        gslot[nd:] = -1
        slotmap[tok_e, which] = gslot
        w_e = wts[tok_e, which]
        core_idx[2 * e] = tok_e[:n1]
        core_wts[2 * e] = w_e[:n1]
        core_idx[2 * e + 1] = tok_e[n1:nd]
        core_wts[2 * e + 1] = w_e[n1:nd]
        if nd < c_e:
            overflow.append((e, tok_e[nd:], w_e[nd:]))
    return core_idx, core_wts, slotmap, overflow


def _feat_major(g):
    """[CAP, D] fp32 -> [NCH, 128, KD, CHW] bf16 (chunk, feature-major)."""
    return np.ascontiguousarray(
        g.reshape(NCH, CHW, KD, 128).transpose(0, 3, 2, 1).astype(np_bf16))


def _prep_inputs(inputs):
    xr2 = np.ascontiguousarray(
        np.asarray(inputs["x_r"], np.float32).reshape(NTOK, D))
    xi2 = np.ascontiguousarray(
        np.asarray(inputs["x_i"], np.float32).reshape(NTOK, D))
    top2, wts = _route(xr2, xi2,
                       np.asarray(inputs["router_w"], np.float32),
                       np.asarray(inputs["router_b"], np.float32))
    core_idx, core_wts, slotmap, overflow = _dispatch(top2, wts)

    # per-expert Karatsuba weight stacks
    def upt(w):  # [DH, D] -> [KH, 128(d), KD, 128(dh)]
        return w.reshape(KH, 128, KD, 128).transpose(0, 3, 2, 1)

    def dnt(w):  # [D, DH] -> [KD, 128(dh), KH, 128(d)]
        return w.reshape(KD, 128, KH, 128).transpose(0, 3, 2, 1)

    ws = {k: np.asarray(inputs[k], np.float32)
          for k in ("ug_wr", "ug_wi", "uv_wr", "uv_wi", "dn_wr", "dn_wi")}

    upw_e, dnw_e = [], []
    for e in range(E):
        ups = [upt(ws["ug_wr"][e]), upt(ws["ug_wi"][e] - ws["ug_wr"][e]),
               upt(ws["ug_wr"][e] + ws["ug_wi"][e]),
               upt(ws["uv_wr"][e]), upt(ws["uv_wi"][e] - ws["uv_wr"][e]),
               upt(ws["uv_wr"][e] + ws["uv_wi"][e])]
        upw_e.append(np.ascontiguousarray(
            np.stack(ups, axis=3).astype(np_bf16)))
        dns = [dnt(ws["dn_wr"][e]), dnt(ws["dn_wi"][e] - ws["dn_wr"][e]),
               dnt(ws["dn_wr"][e] + ws["dn_wi"][e])]
        dnw_e.append(np.ascontiguousarray(
            np.stack(dns, axis=3).astype(np_bf16)))

    xs2 = xr2 + xi2
    in_maps = []
    for c in range(NCORES):
        idx = core_idx[c]
        n_c = len(idx)
        wcol = core_wts[c][:, None]
        gr = np.zeros((CAP, D), np.float32)
        gi = np.zeros((CAP, D), np.float32)
        gs = np.zeros((CAP, D), np.float32)
        vr = np.zeros((CAP, D), np.float32)
        vi = np.zeros((CAP, D), np.float32)
        vs = np.zeros((CAP, D), np.float32)
        gr[:n_c] = xr2[idx]
        gi[:n_c] = xi2[idx]
        gs[:n_c] = xs2[idx]
        vr[:n_c] = xr2[idx] * wcol
        vi[:n_c] = xi2[idx] * wcol
        vs[:n_c] = xs2[idx] * wcol
        in_maps.append({
            "xgr": _feat_major(gr), "xgi": _feat_major(gi),
            "xgs": _feat_major(gs), "xvr": _feat_major(vr),
            "xvi": _feat_major(vi), "xvs": _feat_major(vs),
            "upw": upw_e[c // 2], "dnw": dnw_e[c // 2],
        })
    return in_maps, slotmap, overflow


def _host_expert(inputs, e, idx, w):
    """Exact fp32 expert-e MLP for overflow tokens idx, scaled by w."""
    xr = np.asarray(inputs["x_r"], np.float32).reshape(NTOK, D)[idx]
    xi = np.asarray(inputs["x_i"], np.float32).reshape(NTOK, D)[idx]

    def clin(ar, ai, wr, wi):
        return ar @ wr.T - ai @ wi.T, ai @ wr.T + ar @ wi.T

    ugr = np.asarray(inputs["ug_wr"], np.float32)[e]
    ugi = np.asarray(inputs["ug_wi"], np.float32)[e]
    uvr = np.asarray(inputs["uv_wr"], np.float32)[e]
    uvi = np.asarray(inputs["uv_wi"], np.float32)[e]
    dnr = np.asarray(inputs["dn_wr"], np.float32)[e]
    dni = np.asarray(inputs["dn_wi"], np.float32)[e]
    gr, gi = clin(xr, xi, ugr, ugi)
    mag = np.sqrt(gr * gr + gi * gi + 1e-8)
    gate = mag / (1.0 + np.exp(-mag)) * w[:, None]
    vr, vi = clin(xr, xi, uvr, uvi)
    hr, hi = gate * vr, gate * vi
    yr, yi = clin(hr, hi, dnr, dni)
    return yr, yi


def run(inputs: dict, trace: bool = False):
    """Returns ((out_r, out_i), BassKernelResults)."""
    assert int(inputs["top_k"]) == K, "kernel specialized for top_k=2"
    for bname in ("router_b", "ug_br", "ug_bi", "uv_br", "uv_bi", "dn_br",
                  "dn_bi"):
        assert not np.any(np.asarray(inputs[bname])), \
            f"kernel assumes zero bias ({bname})"

    in_maps, slotmap, overflow = _prep_inputs(inputs)
    nc = _get_nc()
    res = run_bass_kernel_spmd(nc, in_maps, core_ids=list(range(NCORES)),
                               trace=trace)
    # stacked device outputs + a zero row for host-handled (-1) slots
    yr_all = np.zeros((NCORES * CAP + 1, D), np.float32)
    yi_all = np.zeros((NCORES * CAP + 1, D), np.float32)
    for c in range(NCORES):
        sl = slice(c * CAP, (c + 1) * CAP)
        # [NCH, 128, KD, CHW] -> [CAP, D]
        yr_all[sl] = res.results[c]["oyr"].transpose(0, 3, 2, 1).reshape(
            CAP, D)
        yi_all[sl] = res.results[c]["oyi"].transpose(0, 3, 2, 1).reshape(
            CAP, D)
    out_r = yr_all[slotmap[:, 0]] + yr_all[slotmap[:, 1]]
    out_i = yi_all[slotmap[:, 0]] + yi_all[slotmap[:, 1]]
    for e, idx, w in overflow:
        yr, yi = _host_expert(inputs, e, idx, w)
        np.add.at(out_r, idx, yr)
        np.add.at(out_i, idx, yi)
    return (out_r.reshape(B, H, T, D), out_i.reshape(B, H, T, D)), res


def kernel(**inputs):
    (out_r, out_i), _ = run(inputs, trace=False)
    return out_r, out_i


# revision 12
# speedup vs baseline: 1.0225x; 1.0225x over previous
"""Trainium2 Bass kernel for ComplexMoE (E=4 experts, top-2 routing).

Strategy: EXPERT-PARALLEL with host-side token dispatch (per the
sharding hint: "all-to-all dispatch tokens by top-k expert id").
The tiny router (8192x1024x4 matmul + top-2 + softmax) runs on the host
as part of sharding; each of the 8 cores owns ONE expert's weights
(2 cores per expert, each taking up to 2048 of that expert's routed
tokens). A core computes only its ~2048 dispatched token-slots instead
of all 4 experts x 1024 tokens densely: 2x less matmul work, and each
core streams only its own expert's weights. The handful of tokens that
overflow the 2x2048 per-expert device capacity (expected count ~40 of
16384 slots for balanced routing) are computed exactly on the host in
fp32 and added during the combine - capacity is a perf knob, not a
correctness cliff.

Complex matmuls use the 3-multiplication Karatsuba/Gauss form (25%
fewer matmuls); its operand sums are free on the host:
  k1 = W1 @ (ar+ai), k2 = W2 @ ar, k3 = W3 @ ai
  with W1 = wr, W2 = wi-wr, W3 = wr+wi   (host-precomputed)
  yr = k1 - k3 ; yi = k1 + k2
Matmuls run in bf16 (same 1 cycle/row PE rate as f32r, with FWL hiding
the weight loads and half the DMA bytes); accumulation stays fp32 in
PSUM. The top-2 softmax combine weight w folds into the value-path
activations on the host (xv = w*x), so h = silu(|g|) * v_scaled needs
no on-device broadcast or extra multiply:
  y = down(silu(|g(x)|) * v(w*x)) = w * y_expert   (v, down linear).

Per-core device program (SPMD, no collectives, 4 chunks of 512 slots):
  for ch in chunks:
    for j in 8 dh-tiles:   # up: ug and uv complex matmuls
      6 PSUM banks (G1..3,V1..3) <- 24 bf16 matmuls (bank-grouped
      emission so PSUM evacuation never stalls the PE)
      gate: DVE subs/adds from PSUM + ACT square/sqrt/silu ordered so
      neither engine head-of-line blocks; hr/hi/hs written bf16
    for d in 4 d-tiles:    # down complex matmul
      3 PSUM banks <- 24 bf16 matmuls; DVE recombine -> DMA out
"""

import numpy as np
import ml_dtypes

import concourse.bacc as bacc
import concourse.bass as bass
import concourse.mybir as mybir
import concourse.tile as tile
from concourse.bass_utils import run_bass_kernel_spmd

B, H, T, D = 2, 8, 512, 512
DH = 1024
E = 4
K = 2
NCORES = 8
NTOK = B * H * T            # 8192
KD = D // 128               # 4 k-tiles over D
KH = DH // 128              # 8 k-tiles over DH
CHW = 512                   # chunk width (one fp32 psum bank)
NCH = 4
CAP = NCH * CHW             # 2048 device slots per core

f32 = mybir.dt.float32
bf16 = mybir.dt.bfloat16
ACT = mybir.ActivationFunctionType
ALU = mybir.AluOpType
np_bf16 = ml_dtypes.bfloat16


def _build_bass():
    nc = bacc.Bacc(None)

    # gate-path x (raw) and value-path x (combine-weight-scaled),
    # chunk-major feature-major: [NCH, 128, KD, CHW]
    xgr = nc.declare_dram_parameter("xgr", [NCH, 128, KD, CHW], bf16,
                                    isOutput=False)
    xgi = nc.declare_dram_parameter("xgi", [NCH, 128, KD, CHW], bf16,
                                    isOutput=False)
    xgs = nc.declare_dram_parameter("xgs", [NCH, 128, KD, CHW], bf16,
                                    isOutput=False)
    xvr = nc.declare_dram_parameter("xvr", [NCH, 128, KD, CHW], bf16,
                                    isOutput=False)
    xvi = nc.declare_dram_parameter("xvi", [NCH, 128, KD, CHW], bf16,
                                    isOutput=False)
    xvs = nc.declare_dram_parameter("xvs", [NCH, 128, KD, CHW], bf16,
                                    isOutput=False)
    # this core's expert weights, Karatsuba triples stacked innermost:
    # up: (ug_W1, ug_W2, ug_W3, uv_W1, uv_W2, uv_W3); dn: (W1, W2, W3)
    upw = nc.declare_dram_parameter("upw", [KH, 128, KD, 6, 128], bf16,
                                    isOutput=False)
    dnw = nc.declare_dram_parameter("dnw", [KD, 128, KH, 3, 128], bf16,
                                    isOutput=False)
    oyr = nc.declare_dram_parameter("oyr", [NCH, 128, KD, CHW], f32,
                                    isOutput=True)
    oyi = nc.declare_dram_parameter("oyi", [NCH, 128, KD, CHW], f32,
                                    isOutput=True)

    with tile.TileContext(nc) as tc:
        with (
            tc.tile_pool(name="xp", bufs=2) as xp,
            tc.tile_pool(name="hp", bufs=1) as hp,
            tc.tile_pool(name="gt", bufs=2) as gt,
            tc.tile_pool(name="vp", bufs=2) as vp,
            tc.tile_pool(name="wup", bufs=3) as wup,
            tc.tile_pool(name="wdn", bufs=3) as wdn,
            tc.tile_pool(name="outp", bufs=2) as outp,
            tc.tile_pool(name="smalls", bufs=1) as smalls,
            tc.tile_pool(name="psA", bufs=1, space="PSUM") as psA,
            tc.tile_pool(name="psB", bufs=2, space="PSUM") as psB,
        ):
            epsb = smalls.tile([128, 1], f32, tag="epsb")
            nc.vector.memset(epsb, 1e-8)

            def load_x(ch, eng):
                xtr = xp.tile([128, KD, CHW], bf16, tag="xtr")
                xti = xp.tile([128, KD, CHW], bf16, tag="xti")
                xts = xp.tile([128, KD, CHW], bf16, tag="xts")
                xwr = xp.tile([128, KD, CHW], bf16, tag="xwr")
                xwi = xp.tile([128, KD, CHW], bf16, tag="xwi")
                xws = xp.tile([128, KD, CHW], bf16, tag="xws")
                eng.dma_start(out=xts, in_=xgs[ch])
                eng.dma_start(out=xtr, in_=xgr[ch])
                eng.dma_start(out=xti, in_=xgi[ch])
                eng.dma_start(out=xws, in_=xvs[ch])
                eng.dma_start(out=xwr, in_=xvr[ch])
                eng.dma_start(out=xwi, in_=xvi[ch])
                return xtr, xti, xts, xwr, xwi, xws

            cur_x = load_x(0, nc.sync)
            for ch in range(NCH):
                xtr, xti, xts, xwr, xwi, xws = cur_x

                hr = hp.tile([128, KH, CHW], bf16, tag="hr")
                hi = hp.tile([128, KH, CHW], bf16, tag="hi")
                hs = hp.tile([128, KH, CHW], bf16, tag="hs")

                # -------- up projections + gate, per dh-tile --------
                for j in range(KH):
                    uw = wup.tile([128, KD, 6, 128], bf16, tag="uw")
                    nc.sync.dma_start(out=uw, in_=upw[j])
                    G1 = psB.tile([128, CHW], f32, tag="pg1")
                    G2 = psA.tile([128, CHW], f32, tag="pg2")
                    G3 = psA.tile([128, CHW], f32, tag="pg3")
                    V1 = psB.tile([128, CHW], f32, tag="pv1")
                    V2 = psA.tile([128, CHW], f32, tag="pv2")
                    V3 = psA.tile([128, CHW], f32, tag="pv3")
                    # bank-grouped emission: all G matmuls, then all V,
                    # so PSUM evacuation of G overlaps the V matmuls.
                    for k in range(KD):
                        st, sp = (k == 0), (k == KD - 1)
                        nc.tensor.matmul(G1, uw[:, k, 0, :], xts[:, k, :],
                                         start=st, stop=sp)
                        nc.tensor.matmul(G2, uw[:, k, 1, :], xtr[:, k, :],
                                         start=st, stop=sp)
                        nc.tensor.matmul(G3, uw[:, k, 2, :], xti[:, k, :],
                                         start=st, stop=sp)
                    for k in range(KD):
                        st, sp = (k == 0), (k == KD - 1)
                        nc.tensor.matmul(V1, uw[:, k, 3, :], xws[:, k, :],
                                         start=st, stop=sp)
                        nc.tensor.matmul(V2, uw[:, k, 4, :], xwr[:, k, :],
                                         start=st, stop=sp)
                        nc.tensor.matmul(V3, uw[:, k, 5, :], xwi[:, k, :],
                                         start=st, stop=sp)
                    # gate = silu(sqrt(gr^2+gi^2+eps)); h = gate * v
                    g1c = gt.tile([128, CHW], f32, tag="g1c")
                    v1c = gt.tile([128, CHW], f32, tag="v1c")
                    gr = gt.tile([128, CHW], f32, tag="gr")
                    gi = gt.tile([128, CHW], f32, tag="gi")
                    t1 = gt.tile([128, CHW], f32, tag="t1")
                    t2 = gt.tile([128, CHW], f32, tag="t2")
                    t3 = gt.tile([128, CHW], f32, tag="t3")
                    m = gt.tile([128, CHW], f32, tag="m")
                    sm = gt.tile([128, CHW], f32, tag="sm")
                    vr = vp.tile([128, CHW], f32, tag="vr")
                    vi = vp.tile([128, CHW], f32, tag="vi")
                    # DVE reads at most one PSUM operand: stage G1/V1 to
                    # SBUF on the scalar engine. Emission order keeps the
                    # critical chain gr->t1->t2->t3->m->sm->hr/hi/hs
                    # flowing without head-of-line blocks on either queue.
                    nc.scalar.copy(out=g1c, in_=G1)
                    nc.vector.tensor_tensor(gr, g1c, G3, op=ALU.subtract)
                    nc.vector.tensor_tensor(gi, g1c, G2, op=ALU.add)
                    nc.scalar.activation(out=t1, in_=gr, func=ACT.Square)
                    nc.scalar.activation(out=t2, in_=gi, func=ACT.Square)
                    nc.vector.tensor_tensor(t3, t1, t2, op=ALU.add)
                    nc.scalar.copy(out=v1c, in_=V1)
                    nc.scalar.activation(out=m, in_=t3, func=ACT.Sqrt,
                                         bias=epsb)
                    nc.scalar.activation(out=sm, in_=m, func=ACT.Silu)
                    nc.vector.tensor_tensor(vr, v1c, V3, op=ALU.subtract)
                    nc.vector.tensor_tensor(vi, v1c, V2, op=ALU.add)
                    nc.vector.tensor_tensor(hr[:, j, :], sm, vr, op=ALU.mult)
                    nc.vector.tensor_tensor(hi[:, j, :], sm, vi, op=ALU.mult)
                    nc.vector.tensor_tensor(hs[:, j, :], hr[:, j, :],
                                            hi[:, j, :], op=ALU.add)

                # prefetch next chunk's tokens during the down phase on
                # the otherwise-idle GPSIMD queue (SWDGE): its semaphore
                # waits can't head-of-line block any compute there, and
                # the sync ring stays dedicated to the weight stream.
                if ch + 1 < NCH:
                    cur_x = load_x(ch + 1, nc.gpsimd)

                # -------- down projection, per d-tile --------
                for d in range(KD):
                    dw = wdn.tile([128, KH, 3, 128], bf16, tag="dw")
                    nc.sync.dma_start(out=dw, in_=dnw[d])
                    D1 = psB.tile([128, CHW], f32, tag="pg1")
                    D2 = psA.tile([128, CHW], f32, tag="pg2")
                    D3 = psA.tile([128, CHW], f32, tag="pg3")
                    for kh in range(KH):
                        nc.tensor.matmul(D1, dw[:, kh, 0, :], hs[:, kh, :],
                                         start=(kh == 0), stop=(kh == KH - 1))
                    for kh in range(KH):
                        nc.tensor.matmul(D2, dw[:, kh, 1, :], hr[:, kh, :],
                                         start=(kh == 0), stop=(kh == KH - 1))
                    for kh in range(KH):
                        nc.tensor.matmul(D3, dw[:, kh, 2, :], hi[:, kh, :],
                                         start=(kh == 0), stop=(kh == KH - 1))
                    d1c = outp.tile([128, CHW], f32, tag="d1c")
                    yr = outp.tile([128, CHW], f32, tag="yr")
                    yi = outp.tile([128, CHW], f32, tag="yi")
                    nc.scalar.copy(out=d1c, in_=D1)
                    nc.vector.tensor_tensor(yr, d1c, D3, op=ALU.subtract)
                    nc.vector.tensor_tensor(yi, d1c, D2, op=ALU.add)
                    nc.sync.dma_start(out=oyr[ch, :, d, :], in_=yr)
                    nc.scalar.dma_start(out=oyi[ch, :, d, :], in_=yi)
    nc.finalize()
    return nc


_cached_nc = None


def _get_nc():
    global _cached_nc
    if _cached_nc is None:
        _cached_nc = _build_bass()
    return _cached_nc


def _route(xr2, xi2, router_w, router_b):
    """Host router: fp32 logits, stable top-2, softmax weights."""
    logits = (xr2 @ router_w[:, :D].T.astype(np.float32)
              + xi2 @ router_w[:, D:].T.astype(np.float32)
              + router_b[None, :].astype(np.float32))
    order = np.argsort(-logits, axis=1, kind="stable")
    top2 = order[:, :K]                                   # [N, 2]
    l12 = np.take_along_axis(logits, top2, axis=1)
    l12 = l12 - l12[:, :1]                                # max first (sorted)
    ew = np.exp(l12)
    wts = ew / ew.sum(axis=1, keepdims=True)              # [N, 2]
    return top2, wts.astype(np.float32)


def _dispatch(top2, wts):
    """Assign each (token, expert) pair to a device slot or the host.

    Expert e lives on cores 2e and 2e+1 (up to CAP slots each); pairs
    beyond 2*CAP go to the host overflow list (computed exactly in
    numpy). Returns per-core index/weight arrays, the [N, 2] global
    slot map (-1 = host), and the overflow list [(e, tokens, weights)].
    """
    N = top2.shape[0]
    core_idx = [None] * NCORES
    core_wts = [None] * NCORES
    slotmap = np.empty((N, K), np.int64)
    overflow = []
    for e in range(E):
        tok_e, which = np.nonzero(top2 == e)
        c_e = len(tok_e)
        n1 = min((c_e + 1) // 2, CAP)
        n2 = min(c_e - n1, CAP)
        nd = n1 + n2
        pos = np.arange(c_e)
        in2 = pos >= n1
        cores = 2 * e + in2.astype(np.int64)
        pic = np.where(in2, pos - n1, pos)
        gslot = cores * CAP + pic
        gslot[nd:] = -1
        slotmap[tok_e, which] = gslot
        w_e = wts[tok_e, which]
        core_idx[2 * e] = tok_e[:n1]
        core_wts[2 * e] = w_e[:n1]
        core_idx[2 * e + 1] = tok_e[n1:nd]
        core_wts[2 * e + 1] = w_e[n1:nd]
        if nd < c_e:
            overflow.append((e, tok_e[nd:], w_e[nd:]))
    return core_idx, core_wts, slotmap, overflow


def _feat_major(g):
    """[CAP, D] fp32 -> [NCH, 128, KD, CHW] bf16 (chunk, feature-major)."""
    return np.ascontiguousarray(
        g.reshape(NCH, CHW, KD, 128).transpose(0, 3, 2, 1).astype(np_bf16))


def _prep_inputs(inputs):
    xr2 = np.ascontiguousarray(
        np.asarray(inputs["x_r"], np.float32).reshape(NTOK, D))
    xi2 = np.ascontiguousarray(
        np.asarray(inputs["x_i"], np.float32).reshape(NTOK, D))
    top2, wts = _route(xr2, xi2,
                       np.asarray(inputs["router_w"], np.float32),
                       np.asarray(inputs["router_b"], np.float32))
    core_idx, core_wts, slotmap, overflow = _dispatch(top2, wts)

    # per-expert Karatsuba weight stacks
    def upt(w):  # [DH, D] -> [KH, 128(d), KD, 128(dh)]
        return w.reshape(KH, 128, KD, 128).transpose(0, 3, 2, 1)

    def dnt(w):  # [D, DH] -> [KD, 128(dh), KH, 128(d)]
        return w.reshape(KD, 128, KH, 128).transpose(0, 3, 2, 1)

    ws = {k: np.asarray(inputs[k], np.float32)
          for k in ("ug_wr", "ug_wi", "uv_wr", "uv_wi", "dn_wr", "dn_wi")}

    upw_e, dnw_e = [], []
    for e in range(E):
        ups = [upt(ws["ug_wr"][e]), upt(ws["ug_wi"][e] - ws["ug_wr"][e]),
               upt(ws["ug_wr"][e] + ws["ug_wi"][e]),
               upt(ws["uv_wr"][e]), upt(ws["uv_wi"][e] - ws["uv_wr"][e]),
               upt(ws["uv_wr"][e] + ws["uv_wi"][e])]
        upw_e.append(np.ascontiguousarray(
            np.stack(ups, axis=3).astype(np_bf16)))
        dns = [dnt(ws["dn_wr"][e]), dnt(ws["dn_wi"][e] - ws["dn_wr"][e]),
               dnt(ws["dn_wr"][e] + ws["dn_wi"][e])]
        dnw_e.append(np.ascontiguousarray(
            np.stack(dns, axis=3).astype(np_bf16)))

    xs2 = xr2 + xi2
    in_maps = []
    for c in range(NCORES):
        idx = core_idx[c]
        n_c = len(idx)
        wcol = core_wts[c][:, None]
        gr = np.zeros((CAP, D), np.float32)
        gi = np.zeros((CAP, D), np.float32)
        gs = np.zeros((CAP, D), np.float32)
        vr = np.zeros((CAP, D), np.float32)
        vi = np.zeros((CAP, D), np.float32)
        vs = np.zeros((CAP, D), np.float32)
        gr[:n_c] = xr2[idx]
        gi[:n_c] = xi2[idx]
        gs[:n_c] = xs2[idx]
        vr[:n_c] = xr2[idx] * wcol
        vi[:n_c] = xi2[idx] * wcol
        vs[:n_c] = xs2[idx] * wcol
        in_maps.append({
            "xgr": _feat_major(gr), "xgi": _feat_major(gi),
            "xgs": _feat_major(gs), "xvr": _feat_major(vr),
            "xvi": _feat_major(vi), "xvs": _feat_major(vs),
            "upw": upw_e[c // 2], "dnw": dnw_e[c // 2],
        })
    return in_maps, slotmap, overflow


def _host_expert(inputs, e, idx, w):
    """Exact fp32 expert-e MLP for overflow tokens idx, scaled by w."""
    xr = np.asarray(inputs["x_r"], np.float32).reshape(NTOK, D)[idx]
    xi = np.asarray(inputs["x_i"], np.float32).reshape(NTOK, D)[idx]

    def clin(ar, ai, wr, wi):
        return ar @ wr.T - ai @ wi.T, ai @ wr.T + ar @ wi.T

    ugr = np.asarray(inputs["ug_wr"], np.float32)[e]
    ugi = np.asarray(inputs["ug_wi"], np.float32)[e]
    uvr = np.asarray(inputs["uv_wr"], np.float32)[e]
    uvi = np.asarray(inputs["uv_wi"], np.float32)[e]
    dnr = np.asarray(inputs["dn_wr"], np.float32)[e]
    dni = np.asarray(inputs["dn_wi"], np.float32)[e]
    gr, gi = clin(xr, xi, ugr, ugi)
    mag = np.sqrt(gr * gr + gi * gi + 1e-8)
    gate = mag / (1.0 + np.exp(-mag)) * w[:, None]
    vr, vi = clin(xr, xi, uvr, uvi)
    hr, hi = gate * vr, gate * vi
    yr, yi = clin(hr, hi, dnr, dni)
    return yr, yi


def run(inputs: dict, trace: bool = False):
    """Returns ((out_r, out_i), BassKernelResults)."""
    assert int(inputs["top_k"]) == K, "kernel specialized for top_k=2"
    for bname in ("router_b", "ug_br", "ug_bi", "uv_br", "uv_bi", "dn_br",
                  "dn_bi"):
        assert not np.any(np.asarray(inputs[bname])), \
            f"kernel assumes zero bias ({bname})"

    in_maps, slotmap, overflow = _prep_inputs(inputs)
    nc = _get_nc()
    res = run_bass_kernel_spmd(nc, in_maps, core_ids=list(range(NCORES)),
                               trace=trace)
    # stacked device outputs + a zero row for host-handled (-1) slots
    yr_all = np.zeros((NCORES * CAP + 1, D), np.float32)
    yi_all = np.zeros((NCORES * CAP + 1, D), np.float32)
    for c in range(NCORES):
        sl = slice(c * CAP, (c + 1) * CAP)
        # [NCH, 128, KD, CHW] -> [CAP, D]
        yr_all[sl] = res.results[c]["oyr"].transpose(0, 3, 2, 1).reshape(
            CAP, D)
        yi_all[sl] = res.results[c]["oyi"].transpose(0, 3, 2, 1).reshape(
            CAP, D)
    out_r = yr_all[slotmap[:, 0]] + yr_all[slotmap[:, 1]]
    out_i = yi_all[slotmap[:, 0]] + yi_all[slotmap[:, 1]]
    for e, idx, w in overflow:
        yr, yi = _host_expert(inputs, e, idx, w)
        np.add.at(out_r, idx, yr)
        np.add.at(out_i, idx, yi)
    return (out_r.reshape(B, H, T, D), out_i.reshape(B, H, T, D)), res


def kernel(**inputs):
    (out_r, out_i), _ = run(inputs, trace=False)
    return out_r, out_i


# revision 16
# speedup vs baseline: 1.0417x; 1.0187x over previous
"""Trainium2 Bass kernel for ComplexMoE (E=4 experts, top-2 routing).

Strategy: EXPERT-PARALLEL with host-side token dispatch (per the
sharding hint: "all-to-all dispatch tokens by top-k expert id").
The tiny router (8192x1024x4 matmul + top-2 + softmax) runs on the host
as part of sharding; each of the 8 cores owns ONE expert's weights
(2 cores per expert, each taking up to 2048 of that expert's routed
tokens). A core computes only its ~2048 dispatched token-slots instead
of all 4 experts x 1024 tokens densely: 2x less matmul work, and each
core streams only its own expert's weights. The handful of tokens that
overflow the 2x2048 per-expert device capacity (expected count ~40 of
16384 slots for balanced routing) are computed exactly on the host in
fp32 and added during the combine - capacity is a perf knob, not a
correctness cliff.

Complex matmuls use the 3-multiplication Karatsuba/Gauss form (25%
fewer matmuls); its operand sums are free on the host:
  k1 = W1 @ (ar+ai), k2 = W2 @ ar, k3 = W3 @ ai
  with W1 = wr, W2 = wi-wr, W3 = wr+wi   (host-precomputed)
  yr = k1 - k3 ; yi = k1 + k2
Matmuls run in bf16 (same 1 cycle/row PE rate as f32r, with FWL hiding
the weight loads and half the DMA bytes); accumulation stays fp32 in
PSUM. The top-2 softmax combine weight w folds into the value-path
activations on the host (xv = w*x), so h = silu(|g|) * v_scaled needs
no on-device broadcast or extra multiply:
  y = down(silu(|g(x)|) * v(w*x)) = w * y_expert   (v, down linear).

Per-core device program (SPMD, no collectives, 4 chunks of 512 slots):
  for ch in chunks:
    for j in 8 dh-tiles:   # up: ug and uv complex matmuls
      6 PSUM banks (G1..3,V1..3) <- 24 bf16 matmuls (bank-grouped
      emission so PSUM evacuation never stalls the PE)
      gate: DVE subs/adds from PSUM + ACT square/sqrt/silu ordered so
      neither engine head-of-line blocks; hr/hi/hs written bf16
    for d in 4 d-tiles:    # down complex matmul
      3 PSUM banks <- 24 bf16 matmuls; DVE recombine -> DMA out
"""

import numpy as np
import ml_dtypes

import concourse.bacc as bacc
import concourse.bass as bass
import concourse.mybir as mybir
import concourse.tile as tile
from concourse.bass_utils import run_bass_kernel_spmd

B, H, T, D = 2, 8, 512, 512
DH = 1024
E = 4
K = 2
NCORES = 8
NTOK = B * H * T            # 8192
KD = D // 128               # 4 k-tiles over D
KH = DH // 128              # 8 k-tiles over DH
CHW = 512                   # chunk width (one fp32 psum bank)
NCH = 4
CAP = NCH * CHW             # 2048 device slots per core

f32 = mybir.dt.float32
bf16 = mybir.dt.bfloat16
ACT = mybir.ActivationFunctionType
ALU = mybir.AluOpType
np_bf16 = ml_dtypes.bfloat16


def _build_bass():
    nc = bacc.Bacc(None)

    # gate-path x (raw) and value-path x (combine-weight-scaled),
    # chunk-major feature-major: [NCH, 128, KD, CHW]
    xgr = nc.declare_dram_parameter("xgr", [NCH, 128, KD, CHW], bf16,
                                    isOutput=False)
    xgi = nc.declare_dram_parameter("xgi", [NCH, 128, KD, CHW], bf16,
                                    isOutput=False)
    xgs = nc.declare_dram_parameter("xgs", [NCH, 128, KD, CHW], bf16,
                                    isOutput=False)
    xvr = nc.declare_dram_parameter("xvr", [NCH, 128, KD, CHW], bf16,
                                    isOutput=False)
    xvi = nc.declare_dram_parameter("xvi", [NCH, 128, KD, CHW], bf16,
                                    isOutput=False)
    xvs = nc.declare_dram_parameter("xvs", [NCH, 128, KD, CHW], bf16,
                                    isOutput=False)
    # this core's expert weights, Karatsuba triples stacked innermost:
    # up: (ug_W1, ug_W2, ug_W3, uv_W1, uv_W2, uv_W3); dn: (W1, W2, W3)
    upw = nc.declare_dram_parameter("upw", [KH, 128, KD, 6, 128], bf16,
                                    isOutput=False)
    dnw = nc.declare_dram_parameter("dnw", [KD, 128, KH, 3, 128], bf16,
                                    isOutput=False)
    oyr = nc.declare_dram_parameter("oyr", [NCH, 128, KD, CHW], f32,
                                    isOutput=True)
    oyi = nc.declare_dram_parameter("oyi", [NCH, 128, KD, CHW], f32,
                                    isOutput=True)

    with tile.TileContext(nc) as tc:
        with (
            tc.tile_pool(name="xp", bufs=2) as xp,
            tc.tile_pool(name="hp", bufs=1) as hp,
            tc.tile_pool(name="gt", bufs=2) as gt,
            tc.tile_pool(name="vp", bufs=2) as vp,
            tc.tile_pool(name="wup", bufs=3) as wup,
            tc.tile_pool(name="wdn", bufs=4) as wdn,
            tc.tile_pool(name="outp", bufs=2) as outp,
            tc.tile_pool(name="smalls", bufs=1) as smalls,
            tc.tile_pool(name="psA", bufs=1, space="PSUM") as psA,
            tc.tile_pool(name="psB", bufs=2, space="PSUM") as psB,
        ):
            epsb = smalls.tile([128, 1], f32, tag="epsb")
            nc.vector.memset(epsb, 1e-8)

            def load_x(ch, eng):
                xtr = xp.tile([128, KD, CHW], bf16, tag="xtr")
                xti = xp.tile([128, KD, CHW], bf16, tag="xti")
                xts = xp.tile([128, KD, CHW], bf16, tag="xts")
                xwr = xp.tile([128, KD, CHW], bf16, tag="xwr")
                xwi = xp.tile([128, KD, CHW], bf16, tag="xwi")
                xws = xp.tile([128, KD, CHW], bf16, tag="xws")
                eng.dma_start(out=xts, in_=xgs[ch])
                eng.dma_start(out=xtr, in_=xgr[ch])
                eng.dma_start(out=xti, in_=xgi[ch])
                eng.dma_start(out=xws, in_=xvs[ch])
                eng.dma_start(out=xwr, in_=xvr[ch])
                eng.dma_start(out=xwi, in_=xvi[ch])
                return xtr, xti, xts, xwr, xwi, xws

            # chunk-0 x rides the scalar ring so the sync ring can start
            # streaming up-weights immediately, in parallel
            cur_x = load_x(0, nc.scalar)
            for ch in range(NCH):
                xtr, xti, xts, xwr, xwi, xws = cur_x

                hr = hp.tile([128, KH, CHW], bf16, tag="hr")
                hi = hp.tile([128, KH, CHW], bf16, tag="hi")
                hs = hp.tile([128, KH, CHW], bf16, tag="hs")

                # -------- up projections + gate, per dh-tile --------
                dws = []
                for j in range(KH):
                    uw = wup.tile([128, KD, 6, 128], bf16, tag="uw")
                    nc.sync.dma_start(out=uw, in_=upw[j])
                    if j >= 6:
                        # hoist down-weight DMAs so they land before the
                        # down phase starts (sync ring is weights-only)
                        for d in (0, 1) if j == 6 else (2, 3):
                            dw = wdn.tile([128, KH, 3, 128], bf16, tag="dw")
                            nc.sync.dma_start(out=dw, in_=dnw[d])
                            dws.append(dw)
                    G1 = psB.tile([128, CHW], f32, tag="pg1")
                    G2 = psA.tile([128, CHW], f32, tag="pg2")
                    G3 = psA.tile([128, CHW], f32, tag="pg3")
                    V1 = psB.tile([128, CHW], f32, tag="pv1")
                    V2 = psA.tile([128, CHW], f32, tag="pv2")
                    V3 = psA.tile([128, CHW], f32, tag="pv3")
                    # bank-grouped emission: all G matmuls, then all V,
                    # so PSUM evacuation of G overlaps the V matmuls.
                    for k in range(KD):
                        st, sp = (k == 0), (k == KD - 1)
                        nc.tensor.matmul(G1, uw[:, k, 0, :], xts[:, k, :],
                                         start=st, stop=sp)
                        nc.tensor.matmul(G2, uw[:, k, 1, :], xtr[:, k, :],
                                         start=st, stop=sp)
                        nc.tensor.matmul(G3, uw[:, k, 2, :], xti[:, k, :],
                                         start=st, stop=sp)
                    for k in range(KD):
                        st, sp = (k == 0), (k == KD - 1)
                        nc.tensor.matmul(V1, uw[:, k, 3, :], xws[:, k, :],
                                         start=st, stop=sp)
                        nc.tensor.matmul(V2, uw[:, k, 4, :], xwr[:, k, :],
                                         start=st, stop=sp)
                        nc.tensor.matmul(V3, uw[:, k, 5, :], xwi[:, k, :],
                                         start=st, stop=sp)
                    # gate = silu(sqrt(gr^2+gi^2+eps)); h = gate * v
                    g1c = gt.tile([128, CHW], f32, tag="g1c")
                    v1c = gt.tile([128, CHW], f32, tag="v1c")
                    gr = gt.tile([128, CHW], f32, tag="gr")
                    gi = gt.tile([128, CHW], f32, tag="gi")
                    t1 = gt.tile([128, CHW], f32, tag="t1")
                    t2 = gt.tile([128, CHW], f32, tag="t2")
                    t3 = gt.tile([128, CHW], f32, tag="t3")
                    m = gt.tile([128, CHW], f32, tag="m")
                    sm = gt.tile([128, CHW], f32, tag="sm")
                    vr = vp.tile([128, CHW], f32, tag="vr")
                    vi = vp.tile([128, CHW], f32, tag="vi")
                    # DVE reads at most one PSUM operand: stage G1/V1 to
                    # SBUF on the scalar engine. Emission order keeps the
                    # critical chain gr->t1->t2->t3->m->sm->hr/hi/hs
                    # flowing without head-of-line blocks on either queue.
                    nc.scalar.copy(out=g1c, in_=G1)
                    nc.vector.tensor_tensor(gr, g1c, G3, op=ALU.subtract)
                    nc.vector.tensor_tensor(gi, g1c, G2, op=ALU.add)
                    nc.scalar.activation(out=t1, in_=gr, func=ACT.Square)
                    nc.scalar.activation(out=t2, in_=gi, func=ACT.Square)
                    nc.vector.tensor_tensor(t3, t1, t2, op=ALU.add)
                    nc.scalar.copy(out=v1c, in_=V1)
                    nc.scalar.activation(out=m, in_=t3, func=ACT.Sqrt,
                                         bias=epsb)
                    nc.scalar.activation(out=sm, in_=m, func=ACT.Silu)
                    nc.vector.tensor_tensor(vr, v1c, V3, op=ALU.subtract)
                    nc.vector.tensor_tensor(vi, v1c, V2, op=ALU.add)
                    nc.vector.tensor_tensor(hr[:, j, :], sm, vr, op=ALU.mult)
                    nc.vector.tensor_tensor(hi[:, j, :], sm, vi, op=ALU.mult)
                    nc.vector.tensor_tensor(hs[:, j, :], hr[:, j, :],
                                            hi[:, j, :], op=ALU.add)

                # prefetch next chunk's tokens during the down phase on
                # the scalar ring: its x waits are already satisfied when
                # the queue reaches them, and the sync ring stays
                # dedicated to the weight stream.
                if ch + 1 < NCH:
                    cur_x = load_x(ch + 1, nc.scalar)

                # -------- down projection, per d-tile --------
                for d in range(KD):
                    dw = dws[d]
                    D1 = psB.tile([128, CHW], f32, tag="pg1")
                    D2 = psA.tile([128, CHW], f32, tag="pg2")
                    D3 = psA.tile([128, CHW], f32, tag="pg3")
                    for kh in range(KH):
                        nc.tensor.matmul(D1, dw[:, kh, 0, :], hs[:, kh, :],
                                         start=(kh == 0), stop=(kh == KH - 1))
                    for kh in range(KH):
                        nc.tensor.matmul(D2, dw[:, kh, 1, :], hr[:, kh, :],
                                         start=(kh == 0), stop=(kh == KH - 1))
                    for kh in range(KH):
                        nc.tensor.matmul(D3, dw[:, kh, 2, :], hi[:, kh, :],
                                         start=(kh == 0), stop=(kh == KH - 1))
                    d1c = outp.tile([128, CHW], f32, tag="d1c")
                    yr = outp.tile([128, CHW], f32, tag="yr")
                    yi = outp.tile([128, CHW], f32, tag="yi")
                    nc.scalar.copy(out=d1c, in_=D1)
                    nc.vector.tensor_tensor(yr, d1c, D3, op=ALU.subtract)
                    nc.vector.tensor_tensor(yi, d1c, D2, op=ALU.add)
                    nc.scalar.dma_start(out=oyr[ch, :, d, :], in_=yr)
                    nc.scalar.dma_start(out=oyi[ch, :, d, :], in_=yi)
    nc.finalize()
    return nc


_cached_nc = None


def _get_nc():
    global _cached_nc
    if _cached_nc is None:
        _cached_nc = _build_bass()
    return _cached_nc


def _route(xr2, xi2, router_w, router_b):
    """Host router: fp32 logits, stable top-2, softmax weights."""
    logits = (xr2 @ router_w[:, :D].T.astype(np.float32)
              + xi2 @ router_w[:, D:].T.astype(np.float32)
              + router_b[None, :].astype(np.float32))
    order = np.argsort(-logits, axis=1, kind="stable")
    top2 = order[:, :K]                                   # [N, 2]
    l12 = np.take_along_axis(logits, top2, axis=1)
    l12 = l12 - l12[:, :1]                                # max first (sorted)
    ew = np.exp(l12)
    wts = ew / ew.sum(axis=1, keepdims=True)              # [N, 2]
    return top2, wts.astype(np.float32)


def _dispatch(top2, wts):
    """Assign each (token, expert) pair to a device slot or the host.

    Expert e lives on cores 2e and 2e+1 (up to CAP slots each); pairs
    beyond 2*CAP go to the host overflow list (computed exactly in
    numpy). Returns per-core index/weight arrays, the [N, 2] global
    slot map (-1 = host), and the overflow list [(e, tokens, weights)].
    """
    N = top2.shape[0]
    core_idx = [None] * NCORES
    core_wts = [None] * NCORES
    slotmap = np.empty((N, K), np.int64)
    overflow = []
    for e in range(E):
        tok_e, which = np.nonzero(top2 == e)
        c_e = len(tok_e)
        n1 = min((c_e + 1) // 2, CAP)
        n2 = min(c_e - n1, CAP)
        nd = n1 + n2
        pos = np.arange(c_e)
        in2 = pos >= n1
        cores = 2 * e + in2.astype(np.int64)
        pic = np.where(in2, pos - n1, pos)
        gslot = cores * CAP + pic
        gslot[nd:] = -1
        slotmap[tok_e, which] = gslot
        w_e = wts[tok_e, which]
        core_idx[2 * e] = tok_e[:n1]
        core_wts[2 * e] = w_e[:n1]
        core_idx[2 * e + 1] = tok_e[n1:nd]
        core_wts[2 * e + 1] = w_e[n1:nd]
        if nd < c_e:
            overflow.append((e, tok_e[nd:], w_e[nd:]))
    return core_idx, core_wts, slotmap, overflow


def _feat_major(g):
    """[CAP, D] fp32 -> [NCH, 128, KD, CHW] bf16 (chunk, feature-major)."""
    return np.ascontiguousarray(
        g.reshape(NCH, CHW, KD, 128).transpose(0, 3, 2, 1).astype(np_bf16))


def _prep_inputs(inputs):
    xr2 = np.ascontiguousarray(
        np.asarray(inputs["x_r"], np.float32).reshape(NTOK, D))
    xi2 = np.ascontiguousarray(
        np.asarray(inputs["x_i"], np.float32).reshape(NTOK, D))
    top2, wts = _route(xr2, xi2,
                       np.asarray(inputs["router_w"], np.float32),
                       np.asarray(inputs["router_b"], np.float32))
    core_idx, core_wts, slotmap, overflow = _dispatch(top2, wts)

    # per-expert Karatsuba weight stacks
    def upt(w):  # [DH, D] -> [KH, 128(d), KD, 128(dh)]
        return w.reshape(KH, 128, KD, 128).transpose(0, 3, 2, 1)

    def dnt(w):  # [D, DH] -> [KD, 128(dh), KH, 128(d)]
        return w.reshape(KD, 128, KH, 128).transpose(0, 3, 2, 1)

    ws = {k: np.asarray(inputs[k], np.float32)
          for k in ("ug_wr", "ug_wi", "uv_wr", "uv_wi", "dn_wr", "dn_wi")}

    upw_e, dnw_e = [], []
    for e in range(E):
        ups = [upt(ws["ug_wr"][e]), upt(ws["ug_wi"][e] - ws["ug_wr"][e]),
               upt(ws["ug_wr"][e] + ws["ug_wi"][e]),
               upt(ws["uv_wr"][e]), upt(ws["uv_wi"][e] - ws["uv_wr"][e]),
               upt(ws["uv_wr"][e] + ws["uv_wi"][e])]
        upw_e.append(np.ascontiguousarray(
            np.stack(ups, axis=3).astype(np_bf16)))
        dns = [dnt(ws["dn_wr"][e]), dnt(ws["dn_wi"][e] - ws["dn_wr"][e]),
               dnt(ws["dn_wr"][e] + ws["dn_wi"][e])]
        dnw_e.append(np.ascontiguousarray(
            np.stack(dns, axis=3).astype(np_bf16)))

    xs2 = xr2 + xi2
    in_maps = []
    for c in range(NCORES):
        idx = core_idx[c]
        n_c = len(idx)
        wcol = core_wts[c][:, None]
        gr = np.zeros((CAP, D), np.float32)
        gi = np.zeros((CAP, D), np.float32)
        gs = np.zeros((CAP, D), np.float32)
        vr = np.zeros((CAP, D), np.float32)
        vi = np.zeros((CAP, D), np.float32)
        vs = np.zeros((CAP, D), np.float32)
        gr[:n_c] = xr2[idx]
        gi[:n_c] = xi2[idx]
        gs[:n_c] = xs2[idx]
        vr[:n_c] = xr2[idx] * wcol
        vi[:n_c] = xi2[idx] * wcol
        vs[:n_c] = xs2[idx] * wcol
        in_maps.append({
            "xgr": _feat_major(gr), "xgi": _feat_major(gi),
            "xgs": _feat_major(gs), "xvr": _feat_major(vr),
            "xvi": _feat_major(vi), "xvs": _feat_major(vs),
            "upw": upw_e[c // 2], "dnw": dnw_e[c // 2],
        })
    return in_maps, slotmap, overflow


def _host_expert(inputs, e, idx, w):
    """Exact fp32 expert-e MLP for overflow tokens idx, scaled by w."""
    xr = np.asarray(inputs["x_r"], np.float32).reshape(NTOK, D)[idx]
    xi = np.asarray(inputs["x_i"], np.float32).reshape(NTOK, D)[idx]

    def clin(ar, ai, wr, wi):
        return ar @ wr.T - ai @ wi.T, ai @ wr.T + ar @ wi.T

    ugr = np.asarray(inputs["ug_wr"], np.float32)[e]
    ugi = np.asarray(inputs["ug_wi"], np.float32)[e]
    uvr = np.asarray(inputs["uv_wr"], np.float32)[e]
    uvi = np.asarray(inputs["uv_wi"], np.float32)[e]
    dnr = np.asarray(inputs["dn_wr"], np.float32)[e]
    dni = np.asarray(inputs["dn_wi"], np.float32)[e]
    gr, gi = clin(xr, xi, ugr, ugi)
    mag = np.sqrt(gr * gr + gi * gi + 1e-8)
    gate = mag / (1.0 + np.exp(-mag)) * w[:, None]
    vr, vi = clin(xr, xi, uvr, uvi)
    hr, hi = gate * vr, gate * vi
    yr, yi = clin(hr, hi, dnr, dni)
    return yr, yi


def run(inputs: dict, trace: bool = False):
    """Returns ((out_r, out_i), BassKernelResults)."""
    assert int(inputs["top_k"]) == K, "kernel specialized for top_k=2"
    for bname in ("router_b", "ug_br", "ug_bi", "uv_br", "uv_bi", "dn_br",
                  "dn_bi"):
        assert not np.any(np.asarray(inputs[bname])), \
            f"kernel assumes zero bias ({bname})"

    in_maps, slotmap, overflow = _prep_inputs(inputs)
    nc = _get_nc()
    res = run_bass_kernel_spmd(nc, in_maps, core_ids=list(range(NCORES)),
                               trace=trace)
    # stacked device outputs + a zero row for host-handled (-1) slots
    yr_all = np.zeros((NCORES * CAP + 1, D), np.float32)
    yi_all = np.zeros((NCORES * CAP + 1, D), np.float32)
    for c in range(NCORES):
        sl = slice(c * CAP, (c + 1) * CAP)
        # [NCH, 128, KD, CHW] -> [CAP, D]
        yr_all[sl] = res.results[c]["oyr"].transpose(0, 3, 2, 1).reshape(
            CAP, D)
        yi_all[sl] = res.results[c]["oyi"].transpose(0, 3, 2, 1).reshape(
            CAP, D)
    out_r = yr_all[slotmap[:, 0]] + yr_all[slotmap[:, 1]]
    out_i = yi_all[slotmap[:, 0]] + yi_all[slotmap[:, 1]]
    for e, idx, w in overflow:
        yr, yi = _host_expert(inputs, e, idx, w)
        np.add.at(out_r, idx, yr)
        np.add.at(out_i, idx, yi)
    return (out_r.reshape(B, H, T, D), out_i.reshape(B, H, T, D)), res


def kernel(**inputs):
    (out_r, out_i), _ = run(inputs, trace=False)
    return out_r, out_i
